# revision 19
# baseline (speedup 1.0000x reference)
"""DAG-BiNN exact-degree message passing on 8 TRN2 NeuronCores.

Graph: 20000 gene nodes -> 7 levels x 10000 nodes, in-degree 16 per dst.
Per step k: h_dst = tanh(sum_d h_prev[src]*w + bias); head: roots @ W + b.

Sharding: each core owns 1250 dst nodes (padded to 1280) of EVERY level and
the full batch (256). Levels live node-major [rows, 256] bf16 in DRAM; the
per-step gather is a dma_gather(transpose=True) producing batch-major
[128, 2, n_idx] bf16 tiles (split into <=896-idx sub-gathers: the ucode
fails between 896 and 1024 idxs/instr); DVE multiplies by edge weights
(partition-broadcast via DMA) and strided-reduces groups of 16; ACT applies
tanh; PE transposes back to node-major; an AllGather replicates each level
(levels 1..6). Level 7 stays local: each core computes a partial head
matmul [2, 256] and the host sums partials and adds head_b. node_bias adds
are emitted only when node_bias is nonzero (it is all-zeros here).
"""

import os

import numpy as np
import ml_dtypes

# ---- problem constants (hardcoded; kernel.py must be self-contained) ----
GENES = 20000
LEVEL = 10000
STEPS = 7
DEG = 16
B = 256
C = 2
NCORES = 8

PC_D = LEVEL // NCORES          # 1250 real dsts per core per level
CH_DST = 256                    # dsts per chunk
CH = 5                          # chunks per core per level
PC_DP = CH * CH_DST             # 1280 padded dsts per core
SLOT = DEG                      # 16 edge slots per dst
NIDX = CH_DST * SLOT            # 4096 gather indices per chunk
IDXCOL = NIDX // 16             # 256 idx columns (16-partition wrap)
LVL_ROWS = NCORES * PC_DP       # 10240 rows per level table
BHALF = B // 128                # 2 batch halves
SUB_MAX = 896                   # max idxs per dma_gather instruction

BF16 = ml_dtypes.bfloat16

_COMPILED = {}
LAST_RESULT = None  # BassKernelResults of the most recent run (for test.py)


def _lvlrow(pos):
    """Level-k position (0..9999) -> row in the padded level table."""
    return PC_DP * (pos // PC_D) + pos % PC_D


def _prep(inputs):
    """Host-side: build per-core index/weight tables from runtime inputs."""
    X = np.asarray(inputs["X"], np.float32)
    ew = np.asarray(inputs["edge_weight"], np.float32)
    nb = np.asarray(inputs["node_bias"], np.float32)
    hW = np.asarray(inputs["head_W"], np.float32)
    hb = np.asarray(inputs["head_b"], np.float32)
    gm = np.asarray(inputs["gene_map"]).astype(np.int64)
    src = np.asarray(inputs["src"]).astype(np.int64)
    dpos = np.asarray(inputs["dst_pos"]).astype(np.int64)
    du = np.asarray(inputs["dst_unique"]).astype(np.int64)
    eid = np.asarray(inputs["eid"]).astype(np.int64)
    roots = np.asarray(inputs["root_ids"]).astype(np.int64)

    assert X.shape == (B, GENES) and src.shape == (STEPS, LEVEL * DEG)

    # X table: row g holds node gene_map[g]'s batch vector.
    x_tab = np.ascontiguousarray(X.T.astype(BF16))
    row_of_gene = np.empty(GENES, np.int64)
    row_of_gene[gm] = np.arange(GENES)

    max_id = int(du.max()) + 1
    idx_all = np.empty((NCORES, STEPS, CH, 16, IDXCOL), np.int16)
    w_all = np.empty((NCORES, STEPS * CH, NIDX), BF16)
    bias_all = np.zeros((NCORES, STEPS * CH, CH_DST), np.float32)

    lp = np.arange(PC_DP)
    valid = lp < PC_D

    for k in range(STEPS):
        order = np.argsort(dpos[k], kind="stable")
        assert np.all(np.bincount(dpos[k], minlength=LEVEL) == DEG), (
            "kernel assumes exact in-degree 16"
        )
        srcs = src[k][order]                 # [LEVEL*DEG] sorted by dst
        ws = ew[eid[k][order]]

        if k == 0:
            rows = row_of_gene[srcs]
        else:
            pos_of = np.full(max_id, -1, np.int64)
            pos_of[du[k - 1]] = np.arange(LEVEL)
            p = pos_of[srcs]
            assert (p >= 0).all(), "src outside previous level"
            rows = _lvlrow(p)

        rows = rows.reshape(LEVEL, DEG)
        ws = ws.reshape(LEVEL, DEG)
        for c in range(NCORES):
            pos_v = c * PC_D + lp[valid]  # real level positions for this core
            slot_idx = np.zeros((PC_DP, SLOT), np.int64)
            slot_w = np.zeros((PC_DP, SLOT), np.float32)
            slot_idx[valid] = rows[pos_v]
            slot_w[valid] = ws[pos_v]
            bias_all[c, k * CH:(k + 1) * CH] = np.where(
                valid, nb[du[k][np.minimum(lp + c * PC_D, LEVEL - 1)]], 0.0
            ).reshape(CH, CH_DST)
            # chunk + wrap: idx i=(jl2*16+d) -> [i%16, i//16]
            fl = slot_idx.reshape(CH, NIDX).astype(np.int16)
            idx_all[c, k] = fl.reshape(CH, IDXCOL, 16).transpose(0, 2, 1)
            w_all[c, k * CH:(k + 1) * CH] = slot_w.reshape(CH, NIDX).astype(BF16)

    # idx_tab per core: [128, STEPS*CH*IDXCOL], 16-row pattern tiled to 128
    idx16 = idx_all.reshape(NCORES, STEPS * CH, 16, IDXCOL)
    idx16 = idx16.transpose(0, 2, 1, 3).reshape(NCORES, 16, STEPS * CH * IDXCOL)
    idx_tab = np.tile(idx16, (1, NCORES, 1))  # [NCORES, 128, S*CH*IDXCOL]

    # head: W_eff[node] = sum of head_W rows whose root_ids hit that node
    W_eff = np.zeros((max_id, C), np.float32)
    np.add.at(W_eff, np.minimum(roots, max_id - 1), hW)
    head_tabs = []
    for c in range(NCORES):
        Wc = np.zeros((PC_DP, C), np.float32)
        Wc[valid] = W_eff[du[STEPS - 1][c * PC_D + lp[valid]]]
        head_tabs.append(
            np.ascontiguousarray(
                Wc.reshape(CH * BHALF, 128, C).transpose(1, 0, 2)
            ).astype(BF16)
        )  # [128, 10, 2]

    has_bias = bool(np.any(nb != 0.0))
    in_maps = []
    for c in range(NCORES):
        m = {
            "x_tab": x_tab,
            "idx_tab": np.ascontiguousarray(idx_tab[c]),
            "w_tab": np.ascontiguousarray(w_all[c]),
            "head_w": head_tabs[c],
        }
        if has_bias:
            m["bias_tab"] = np.ascontiguousarray(bias_all[c])
        in_maps.append(m)
    return in_maps, hb, has_bias


def _build_nc(has_bias):
    import concourse.bacc as bacc
    import concourse.mybir as mybir
    import concourse.tile as tile
    from concourse.masks import make_identity

    f32 = mybir.dt.float32
    bf16 = mybir.dt.bfloat16
    i16 = mybir.dt.int16

    nc = bacc.Bacc(num_devices=NCORES)
    x_tab = nc.declare_dram_parameter("x_tab", [GENES, B], bf16, isOutput=False)
    idx_tab = nc.declare_dram_parameter(
        "idx_tab", [128, STEPS * CH * IDXCOL], i16, isOutput=False
    )
    w_tab = nc.declare_dram_parameter(
        "w_tab", [STEPS * CH, NIDX], bf16, isOutput=False
    )
    head_w = nc.declare_dram_parameter(
        "head_w", [128, CH * BHALF, C], bf16, isOutput=False
    )
    bias_tab = None
    if has_bias:
        bias_tab = nc.declare_dram_parameter(
            "bias_tab", [STEPS * CH, CH_DST], f32, isOutput=False
        )
    out_partial = nc.declare_dram_parameter("out_partial", [C, B], f32, isOutput=True)

    # level tables (gather sources); collective output should be Shared
    lvl = [
        nc.dram_tensor(f"lvl{i}", [LVL_ROWS, B], bf16, addr_space="Shared")
        for i in range(2)
    ]
    own_slice = nc.dram_tensor("own_slice", [PC_DP, B], bf16)

    # sub-gather split: <=896 idxs per instruction, multiples of 128
    subs = []
    off = 0
    while off < NIDX:
        ni = min(SUB_MAX, NIDX - off)
        subs.append((off, ni))
        off += ni

    with tile.TileContext(nc) as tc:
        with (
            tc.tile_pool(name="const", bufs=1) as const_pool,
            tc.tile_pool(name="msg", bufs=3) as msg_pool,
            tc.tile_pool(name="wsb", bufs=2) as w_pool,
            tc.tile_pool(name="agg", bufs=2) as agg_pool,
            tc.tile_pool(name="hch", bufs=2) as h_pool,
            tc.tile_pool(name="ps", bufs=4, space="PSUM") as psum_pool,
            tc.tile_pool(name="pshead", bufs=1, space="PSUM") as psum_head,
        ):
            # --- persistent setup ---
            idx_sb = const_pool.tile([128, STEPS * CH * IDXCOL], i16)
            nc.sync.dma_start(out=idx_sb[:], in_=idx_tab[:, :])
            hw_sb = const_pool.tile([128, CH * BHALF, C], bf16)
            nc.sync.dma_start(out=hw_sb[:], in_=head_w[:, :, :])
            ident = const_pool.tile([128, 128], bf16)
            make_identity(nc, ident[:])
            nm_sb = const_pool.tile([128, CH * BHALF, B], bf16)
            sub_regs = {ni: nc.gpsimd.to_reg(ni) for ni in {s[1] for s in subs}}

            for k in range(STEPS):
                srctab = x_tab if k == 0 else lvl[(k - 1) % 2]
                for u in range(CH):
                    r = k * CH + u
                    w_sb = w_pool.tile([128, NIDX], bf16)
                    nc.sync.dma_start(
                        out=w_sb[:], in_=w_tab[r : r + 1, :].to_broadcast([128, NIDX])
                    )
                    agg = agg_pool.tile([128, BHALF, CH_DST], f32)
                    for i0, ni in subs:
                        # exact-size tile: gather out must be contiguous
                        msg = msg_pool.tile([128, BHALF, ni], bf16, tag="msg")
                        nc.gpsimd.dma_gather(
                            out_ap=msg[:],
                            in_ap=srctab[:, :],
                            idxs_ap=idx_sb[
                                :,
                                r * IDXCOL + i0 // 16 : r * IDXCOL + (i0 + ni) // 16,
                            ],
                            num_idxs=ni,
                            num_idxs_reg=sub_regs[ni],
                            elem_size=B,
                            transpose=True,
                        )
                        for jb in range(BHALF):
                            nc.vector.tensor_tensor(
                                out=msg[:, jb, :],
                                in0=msg[:, jb, :],
                                in1=w_sb[:, i0 : i0 + ni],
                                op=mybir.AluOpType.mult,
                            )
                        nd = ni // SLOT
                        j0 = i0 // SLOT
                        nc.vector.reduce_sum(
                            out=agg[:, :, j0 : j0 + nd],
                            in_=msg[:].rearrange("p a (j d) -> p a j d", d=SLOT),
                            axis=mybir.AxisListType.X,
                        )
                    if has_bias:
                        bias_bc = w_pool.tile([128, CH_DST], f32, tag="biasbc")
                        nc.sync.dma_start(
                            out=bias_bc[:],
                            in_=bias_tab[r : r + 1, :].to_broadcast([128, CH_DST]),
                        )
                        for jb in range(BHALF):
                            nc.vector.tensor_tensor(
                                out=agg[:, jb, :],
                                in0=agg[:, jb, :],
                                in1=bias_bc[:],
                                op=mybir.AluOpType.add,
                            )
                    hch = h_pool.tile([128, BHALF, CH_DST], bf16)
                    nc.scalar.activation(
                        out=hch[:], in_=agg[:], func=mybir.ActivationFunctionType.Tanh
                    )
                    # transpose [batch, dst] -> [dst, batch] via PE
                    for jb in range(BHALF):
                        for t2 in range(CH_DST // 128):
                            pt = psum_pool.tile([128, 128], bf16)
                            nc.tensor.transpose(
                                out=pt[:],
                                in_=hch[:, jb, t2 * 128 : (t2 + 1) * 128],
                                identity=ident[:],
                            )
                            nc.scalar.copy(
                                out=nm_sb[
                                    :, u * 2 + t2, jb * 128 : (jb + 1) * 128
                                ],
                                in_=pt[:],
                            )
                if k < STEPS - 1:
                    # publish level k+1: own 1280 rows -> all-gather to lvl[k%2]
                    # own_slice row (t*128+p) <- nm_sb[p, t, :]
                    nc.sync.dma_start(
                        out=own_slice[:, :].rearrange("(t p) b -> p t b", p=128),
                        in_=nm_sb[:],
                    )
                    nc.gpsimd.collective_compute(
                        "AllGather",
                        mybir.AluOpType.bypass,
                        replica_groups=[list(range(NCORES))],
                        ins=[own_slice[:, :]],
                        outs=[lvl[k % 2][:, :]],
                    )
                else:
                    pm = psum_head.tile([C, B], f32)
                    nt = CH * BHALF
                    for t in range(nt):
                        nc.tensor.matmul(
                            out=pm[:],
                            lhsT=hw_sb[:, t, :],
                            rhs=nm_sb[:, t, :],
                            start=(t == 0),
                            stop=(t == nt - 1),
                        )
                    res = const_pool.tile([C, B], f32)
                    nc.vector.tensor_copy(out=res[:], in_=pm[:])
                    nc.sync.dma_start(out=out_partial[:, :], in_=res[:])
    nc.finalize()
    return nc


def kernel(**inputs):
    global LAST_RESULT
    from concourse.bass_utils import run_bass_kernel_spmd

    in_maps, hb, has_bias = _prep(inputs)

    key = ("nc", has_bias)
    if key not in _COMPILED:
        _COMPILED[key] = _build_nc(has_bias)
    nc = _COMPILED[key]

    trace = os.environ.get("BASS_TRACE", "0") == "1"
    res = run_bass_kernel_spmd(
        nc, in_maps, core_ids=list(range(NCORES)), trace=trace
    )
    LAST_RESULT = res

    partials = np.stack(
        [np.asarray(r["out_partial"], np.float32) for r in res.results]
    )
    out = partials.sum(axis=0).T + hb[None, :]
    return out.astype(np.float32)
